# revision 40
# baseline (speedup 1.0000x reference)
"""Trainium2 Bass kernel for nn_NeuralControlActor (batch-1 MLP + 4x Mamba2 + MLP).

Tensor-parallel over 8 NeuronCores:
  - mlp_in W1/W2 row-sharded (+AllGather), Mamba2 heads sharded 4/core
    (B/C/conv replicated), gated-RMSNorm stat + Wout partial fused into one
    AllReduce per layer, mlp_out w3/w4 row-sharded (+AllGather).
  - The T=32 sequential scan is computed in closed form (SSD):
    Y_h = (exp(S_t - S_s) o (B^T C) o dt_s + D_h I)^T @ X_h  -- all matmuls,
    batched across the 4 local heads via broadcast matmuls ([32, 128] tiles).
  - Big weights stream as fp8e4 (host-cast); SSD/norm math stays f32.
  - Perf: reciprocal_approx_fast for all sigmoid/silu denominators; depthwise
    conv as batched [128,128] tap ops with host-broadcast weights; the rms
    scale of layer l is folded into layer l+1's elementwise ops so the next
    layer's matmuls read the AllReduce result (comb2) directly; stage D runs
    fp8 DoubleRow with a column-major [128,16] output and batched sigmoid;
    dummy PE matmuls keep the tensor-engine p-state up across collectives;
    full-depth weight pools so w2/w3 prefetch entirely under the NRT entry
    barrier / layer ARs.

Self-contained: hardcodes all shapes; host prep is pure numpy.
"""
import os
import numpy as np
import ml_dtypes

# 66KB AllReduces pick RDH (3-stage) by default; Mesh is lower-latency at
# this size.  Must be set before the NRT builds its communicator.
os.environ.setdefault("NEURON_RT_DBG_RDH_CC", "0")

S, H, T, NL = 4096, 1024, 32, 4
DI, DS, DC, HD, NH = 2048, 128, 4, 64, 32
CD = DI + 2 * DS
DIP = 2 * DI + 2 * DS + NH
ASIZE = 16384
NCORES = 8
HC = NH // NCORES            # 4 heads/core
XC = HC * HD                 # 256 x/z channels per core
MASK_POS = 1000.0
BF16 = ml_dtypes.bfloat16
FP8 = ml_dtypes.float8_e4m3

# ----------------------------------------------------------------- host prep

def _chunkT(w):
    """w [F, K] -> arr [128, K//128, F]: arr[p, k, f] = w[f, k*128+p]."""
    F, K = w.shape
    kc = K // 128
    return np.ascontiguousarray(w.T.reshape(kc, 128, F).transpose(1, 0, 2))


def _prep_consts():
    f32 = np.float32
    c = {}
    ut = np.triu(np.ones((T, T), f32))
    c["negut"] = np.ascontiguousarray(-ut)
    c["maskpos"] = np.ascontiguousarray((np.tril(np.ones((T, T), f32), -1)
                                         * MASK_POS).astype(f32))
    oh = np.zeros((HC, HC * T), f32)
    for h in range(HC):
        oh[h, h * T:(h + 1) * T] = 1.0
    c["onehots"] = oh
    c["ones1"] = np.ones((1, 128), f32)
    # -maskpos replicated per head block: [T, HC*T]
    c["maskrep"] = np.ascontiguousarray(
        np.tile(-c["maskpos"], (1, HC)))
    return c


def _prep_core(inp, c, wdt):
    f32 = np.float32
    m = {}
    x = np.asarray(inp["x"], f32)
    m["x_sb"] = np.ascontiguousarray(x.reshape(32, 128).T).astype(wdt)

    w1c = np.asarray(inp["w1"], f32)[c * 128:(c + 1) * 128]
    m["w1t"] = _chunkT(w1c).astype(FP8)                          # [128,32,128]
    m["b1row"] = np.asarray(inp["b1"], f32)[None, c * 128:(c + 1) * 128]

    w2c = np.asarray(inp["w2"], f32)[c * 4096:(c + 1) * 4096]
    w2t = _chunkT(w2c)                                           # [128,8,4096]
    m["w2t"] = np.ascontiguousarray(
        w2t.reshape(128, 8, 8, 512).transpose(2, 0, 1, 3)).astype(FP8)
    m["b2row"] = np.asarray(inp["b2"], f32)[None, c * 4096:(c + 1) * 4096]

    for l in range(NL):
        win = np.asarray(inp["m_Win"], f32)[l]
        zrows = win[c * XC:(c + 1) * XC]
        dtrows = win[2 * DI + 2 * DS + c * HC:2 * DI + 2 * DS + (c + 1) * HC]
        m[f"winzd{l}"] = _chunkT(np.concatenate([zrows, dtrows], 0)).astype(FP8)
        xrows = win[DI + c * XC:DI + (c + 1) * XC]
        brows = win[2 * DI:2 * DI + DS]
        crows = win[2 * DI + DS:2 * DI + 2 * DS]
        m[f"winxbc{l}"] = _chunkT(
            np.concatenate([xrows, brows, crows], 0)).astype(FP8)  # [128,8,512]

        cw = np.asarray(inp["m_convw"], f32)[l]
        cb = np.asarray(inp["m_convb"], f32)[l]
        chsel = np.concatenate([
            np.arange(c * XC, (c + 1) * XC),
            np.arange(DI, DI + DS),
            np.arange(DI + DS, DI + 2 * DS)])
        cwp = cw[chsel].reshape(4, 128, 4).transpose(1, 0, 2)    # [p, ft, k]
        # broadcast tap weights over tokens: [128, k, ft, T] -> [128, 4*128]
        cwb = np.broadcast_to(
            cwp.transpose(0, 2, 1)[:, :, :, None],
            (128, DC, 4, T)).reshape(128, DC * 4 * T)
        m[f"cwb{l}"] = np.ascontiguousarray(cwb)                 # [128, 512]
        cbb = np.broadcast_to(
            cb[chsel].reshape(4, 128).T[:, :, None],
            (128, 4, T)).reshape(128, 4 * T)
        m[f"cbb{l}"] = np.ascontiguousarray(cbb)                 # [128, 128]

        dtb = np.asarray(inp["m_dtbias"], f32)[l, c * HC:(c + 1) * HC]
        m[f"dtbias{l}"] = np.broadcast_to(dtb[None, :], (T, HC)).copy()
        A = -np.exp(np.asarray(inp["m_Alog"], f32)[l, c * HC:(c + 1) * HC])
        m[f"abc{l}"] = np.broadcast_to(A[None, :], (T, HC)).copy()
        Dv = np.asarray(inp["m_D"], f32)[l, c * HC:(c + 1) * HC]
        dd = np.zeros((T, HC, T), f32)
        for h in range(HC):
            dd[:, h, :] = np.eye(T, dtype=f32) * Dv[h]
        m[f"ddiag{l}"] = dd

        woutp = (np.asarray(inp["m_Wout"], f32)[l]
                 * np.asarray(inp["m_normw"], f32)[l][None, :])
        wc = woutp[:, c * XC:(c + 1) * XC]
        m[f"wout{l}"] = np.ascontiguousarray(
            wc.T.reshape(2, 128, 8, 128).transpose(1, 0, 2, 3)).astype(FP8)

    w3c = np.asarray(inp["w3"], f32)[c * 128:(c + 1) * 128]
    w3t = _chunkT(w3c)                                           # [128,256,128]
    m["w3dr"] = np.ascontiguousarray(
        w3t.reshape(128, 8, 32, 128).transpose(1, 0, 2, 3)).astype(FP8)
    m["b3row"] = np.asarray(inp["b3"], f32)[None, c * 128:(c + 1) * 128]
    w4c = np.asarray(inp["w4"], f32)[c * 2048:(c + 1) * 2048]
    # stationary-weights DoubleRow: w4dr[nt, p, c4, pr, i, j]
    #   = w4c[(4nt+c4)*128 + j, (2pr+i)*128 + p]
    w4v = w4c.reshape(16, 128, 8, 128)            # [c, j, kc, p]
    w4v = w4v.reshape(16, 128, 4, 2, 128)         # [c, j, pr, i, p]
    w4dr = w4v.transpose(4, 0, 2, 3, 1)           # [p, c, pr, i, j]
    w4dr = w4dr.reshape(128, 4, 4, 4, 2, 128).transpose(1, 0, 2, 3, 4, 5)
    m["w4dr"] = np.ascontiguousarray(w4dr).astype(FP8)
    m["b4row"] = np.asarray(inp["b4"], f32)[None, c * 2048:(c + 1) * 2048]
    m["b3col"] = np.ascontiguousarray(
        np.asarray(inp["b3"], f32)[c * 128:(c + 1) * 128][:, None])
    m["b4mat"] = np.ascontiguousarray(
        np.asarray(inp["b4"], f32)[c * 2048:(c + 1) * 2048]
        .reshape(16, 128).T)                      # [p, cblk]

    # packed consts: g32 [32, W], g1 [1, W], gcw [128, W]
    cst = _prep_consts()
    g32 = [cst["negut"], cst["maskpos"],
           np.concatenate([cst["onehots"],
                           np.zeros((T - HC, HC * T), f32)], 0),
           cst["maskrep"]]
    for l in range(NL):
        g32 += [m.pop(f"dtbias{l}"), m.pop(f"abc{l}"),
                m.pop(f"ddiag{l}").reshape(T, HC * T)]
    m["g32"] = np.concatenate(g32, 1)                       # [32, 832]
    g1 = [cst["ones1"], m.pop("b1row"), m.pop("b3row"),
          m.pop("b2row"), m.pop("b4row")]
    m["g1"] = np.concatenate(g1, 1)                         # [1, 6528]
    gcw = []
    for l in range(NL):
        gcw += [m.pop(f"cwb{l}"), m.pop(f"cbb{l}")]
    gcw += [m.pop("b3col"), m.pop("b4mat")]
    m["gcw"] = np.concatenate(gcw, 1)                       # [128, 2577]
    return m


# ------------------------------------------------------------- bass program

def _build_program(wdt_np, collectives=True):
    from contextlib import ExitStack
    import concourse.bacc as bacc
    import concourse.tile as tile
    import concourse.bass as bass
    from concourse import mybir
    from concourse.masks import make_identity

    f32 = mybir.dt.float32
    wdt = mybir.dt.from_np(np.dtype(wdt_np))
    dt8 = mybir.dt.float8e4
    Alu = mybir.AluOpType
    Act = mybir.ActivationFunctionType

    # Force a single ACT table set: every function this kernel uses
    # (exp, ln, relu, square, copy) lives in natural_log_exp_and_others;
    # the default chooser thrashes between exp-only and ln-only sets.
    if not getattr(bacc, "_act_tables_pinned", False):
        _orig_gat = bacc.get_activation_tables

        def _gat(arch):
            t = _orig_gat(arch)
            keep = "natural_log_exp_and_others"
            if keep in t:
                for k in t:
                    if k != keep:
                        t[k] = set()
            return t

        bacc.get_activation_tables = _gat
        bacc._act_tables_pinned = True

    nc = bacc.Bacc("TRN2", target_bir_lowering=False, debug=False,
                   num_devices=NCORES)

    def din(name, shape, dt=wdt):
        return nc.dram_tensor(name, list(shape), dt, kind="ExternalInput").ap()

    # inputs (names must match the per-core map)
    x_sb_d = din("x_sb", [128, 32])
    w1t_d = din("w1t", [128, 32, 128], dt8)
    w2t_d = din("w2t", [8, 128, 8, 512], dt8)
    layer_d = []
    for l in range(NL):
        layer_d.append(dict(
            winzd=din(f"winzd{l}", [128, 8, 260], dt8),
            winxbc=din(f"winxbc{l}", [128, 8, 512], dt8),
            wout=din(f"wout{l}", [128, 2, 8, 128], dt8),
        ))
    g32_d = din("g32", [T, 864], f32)
    g1_d = din("g1", [1, 6528], f32)
    gcw_d = din("gcw", [128, 2577], f32)
    w3dr_d = din("w3dr", [8, 128, 32, 128], dt8)
    w4dr_d = din("w4dr", [4, 128, 4, 4, 2, 128], dt8)
    out_d = nc.dram_tensor("out", [128, 16], f32, kind="ExternalOutput").ap()

    RG = [list(range(NCORES))]

    def _collective(kind, op, ins, outs):
        if collectives:
            nc.gpsimd.collective_compute(kind, op, replica_groups=RG,
                                         ins=ins, outs=outs)
        else:
            nc.sync.dma_start(out=outs[0][0:1], in_=ins[0][0:1])

    from concourse.tile import add_dep_helper

    with tile.TileContext(nc) as tc, ExitStack() as ctx:
        ep = ctx.enter_context
        consts = ep(tc.tile_pool(name="consts", bufs=1))
        pw1 = ep(tc.tile_pool(name="pw1", bufs=1))
        pw2 = ep(tc.tile_pool(name="pw2", bufs=8))
        pwin = ep(tc.tile_pool(name="pwin", bufs=2))
        pw3 = ep(tc.tile_pool(name="pw3", bufs=8))
        pw4 = ep(tc.tile_pool(name="pw4", bufs=4))
        pact = ep(tc.tile_pool(name="pact", bufs=2))
        pact1 = ep(tc.tile_pool(name="pact1", bufs=1))
        psm = ep(tc.tile_pool(name="psm", bufs=2))
        pmv = ep(tc.tile_pool(name="pmv", bufs=2, space="PSUM"))
        pcol = ep(tc.tile_pool(name="pcol", bufs=1, space="PSUM"))
        pps = ep(tc.tile_pool(name="pps", bufs=2, space="PSUM"))
        ppt = ep(tc.tile_pool(name="ppt", bufs=2, space="PSUM"))
        pyacc = ep(tc.tile_pool(name="pyacc", bufs=1, space="PSUM"))
        dram = ep(tc.tile_pool(name="dram", bufs=2, space="DRAM"))

        def sb(pool, shape, dt=f32, tag=None):
            return pool.tile(list(shape), dt, tag=tag, name=tag)

        # ---- constants into SBUF (x/w1 first: stage A is the launch path)
        x_sb = sb(consts, [128, 32], wdt, tag="x_sb")
        nc.sync.dma_start(out=x_sb, in_=x_sb_d)
        w1sb = sb(pw1, [128, 32, 128], dt8, tag="w1")
        nc.sync.dma_start(out=w1sb, in_=w1t_d)
        idn = sb(consts, [128, 128], f32, tag="idn")
        make_identity(nc, idn)
        eps_t = sb(consts, [128, 1], f32, tag="eps_t")
        nc.vector.memset(eps_t, 1e-5)
        g32 = sb(consts, [T, 864], f32, tag="g32")
        nc.sync.dma_start(out=g32, in_=g32_d)
        g1 = sb(consts, [1, 6528], f32, tag="g1")
        nc.sync.dma_start(out=g1, in_=g1_d)
        gcw = sb(consts, [128, 2577], f32, tag="gcw")
        nc.sync.dma_start(out=gcw, in_=gcw_d)
        b3col = gcw[:, 2560:2561]
        b4mat = gcw[:, 2561:2577]
        negut = g32[:, 0:T]
        maskpos = g32[:, T:2 * T]
        onehots = g32[0:HC, 2 * T:2 * T + HC * T]
        maskrep = g32[:, 2 * T + HC * T:2 * T + 2 * HC * T]
        ones1 = g1[0:1, 0:128]
        b1row = g1[0:1, 128:256]
        b3row = g1[0:1, 256:384]
        b2row = g1[0:1, 384:4480]
        b4row = g1[0:1, 4480:6528]
        G32L = 2 * T + 2 * HC * T        # per-layer base in g32
        lc = []
        for l in range(NL):
            base = G32L + l * (2 * HC + HC * T)
            cb = l * 640
            lc.append(dict(
                dtbias=g32[:, base:base + HC],
                abc=g32[:, base + HC:base + 2 * HC],
                ddiag=g32[:, base + 2 * HC:base + 2 * HC + HC * T],
                cwb=[gcw[:, cb + k * 128:cb + (k + 1) * 128] for k in range(4)],
                cbb=gcw[:, cb + 512:cb + 640],
            ))
        # SSD broadcast-matmul helper: ones [4, 32] for row-broadcast
        ones4 = sb(consts, [HC, T], f32, tag="ones4")
        nc.vector.memset(ones4, 1.0)

        # PE warmup bursts: keep HAM busy (and the PE p-state high) while
        # DMAs / collectives are in flight.  Rotates the shared t128 bufs.
        def warm(n):
            for _ in range(n):
                wp = pps.tile([128, 128], f32, tag="t128", name="t128")
                nc.tensor.matmul(wp, idn, idn[:, 0:128],
                                 start=True, stop=True)

        warm(24)

        # ---- stage A: h = relu(W1 x + b1), row shard -> AllGather
        ps_h = pmv.tile([1, 512], f32, tag="mv", name="mv")
        for j in range(32):
            nc.tensor.matmul(ps_h[0:1, 0:128], x_sb[:, j:j + 1], w1sb[:, j, :],
                             start=(j == 0), stop=(j == 31))
        h_tmp = sb(pact, [1, 128], f32, tag="h_tmp")
        i_gate_a = nc.vector.tensor_tensor(h_tmp, ps_h[0:1, 0:128], b1row, Alu.add)
        h_act = sb(pact, [1, 128], f32, tag="h_act")
        nc.scalar.activation(h_act, h_tmp, Act.Relu)
        hin = dram.tile([1, 128], f32, tag="hin", name="hin")
        nc.sync.dma_start(out=hin[:], in_=h_act)
        hout = dram.tile([NCORES, 128], f32, tag="hout", name="hout")
        _collective("AllGather", Alu.bypass, [hin[:].opt()], [hout[:].opt()])
        h_r = sb(pact, [NCORES, 128], f32, tag="h_r")
        nc.sync.dma_start(out=h_r, in_=hout[:])
        ps_ht = pps.tile([128, 128], f32, tag="t128", name="t128")
        nc.tensor.transpose(ps_ht[:, 0:NCORES], h_r, idn[0:NCORES, 0:NCORES])
        h_all = sb(pact, [128, NCORES], wdt, tag="h_all")
        nc.vector.tensor_copy(out=h_all, in_=ps_ht[:, 0:NCORES])

        # ---- stage B: seq shard = W2 h + b2 (4 tokens) -> AllGather
        seq_sb = sb(pact1, [1, 4096], f32, tag="seq_sb")
        for nt in range(8):
            w2sb = sb(pw2, [128, 8, 512], dt8, tag="w2")
            i_d = nc.scalar.dma_start(out=w2sb, in_=w2t_d[nt])
            add_dep_helper(i_d.ins, i_gate_a.ins, reason="pace w2 after stage A")
            ps = pmv.tile([1, 512], f32, tag="mv", name="mv")
            for j in range(8):
                nc.tensor.matmul(ps, h_all[:, j:j + 1], w2sb[:, j, :],
                                 start=(j == 0), stop=(j == 7))
            nc.vector.tensor_tensor(seq_sb[0:1, nt * 512:(nt + 1) * 512], ps,
                                    b2row[0:1, nt * 512:(nt + 1) * 512], Alu.add)
        seqin = dram.tile([1, 4096], f32, tag="seqin", name="seqin")
        nc.sync.dma_start(out=seqin[:], in_=seq_sb)
        seqout = dram.tile([NCORES, 4096], f32, tag="seqout", name="seqout")
        _collective("AllGather", Alu.bypass, [seqin[:].opt()], [seqout[:].opt()])
        warm(14)

        # ---- layer-0 input: load [32,1024] token-major, PE-transpose to uT
        useq = sb(pact1, [T, 1024], f32, tag="useq")
        i_gate_l0 = nc.sync.dma_start(
            out=useq, in_=seqout[:].rearrange("j (a f) -> (j a) f", a=4))
        uTb = sb(pact, [128, 8, T], wdt, tag="uTb")
        for k in range(8):
            pt = pps.tile([128, T], f32, tag="t128", name="t128")
            nc.tensor.transpose(pt, useq[:, k * 128:(k + 1) * 128],
                                idn[0:T, 0:T])
            nc.vector.tensor_copy(out=uTb[:, k, :], in_=pt)

        # ---- Mamba2 layers
        prev_c2 = None
        for l in range(NL):
            ld, cl = layer_d[l], lc[l]
            winzd = sb(pwin, [128, 8, 260], dt8, tag="winzd")
            i_d = nc.gpsimd.dma_start(out=winzd, in_=ld["winzd"])
            add_dep_helper(i_d.ins, i_gate_a.ins, reason="pace win after stage A")
            winxbc = sb(pwin, [128, 8, 512], dt8, tag="winxbc")
            i_d = nc.gpsimd.dma_start(out=winxbc, in_=ld["winxbc"])
            add_dep_helper(i_d.ins, i_gate_a.ins, reason="pace win after stage A")
            woutsb = sb(pwin, [128, 2, 8, 128], dt8, tag="wout")
            i_d = nc.gpsimd.dma_start(out=woutsb, in_=ld["wout"])
            add_dep_helper(i_d.ins, i_gate_a.ins, reason="pace win after stage A")

            if l > 0:
                # rms scale of the previous layer's output, folded downstream
                s_col = sb(pact, [T, 1], f32, tag="s_col")
                nc.scalar.activation(s_col, prev_c2[0:T, 256:257], Act.Ln,
                                     bias=eps_t[0:T], scale=1.0 / DI)
                r_col = sb(pact, [T, 1], f32, tag="r_col")
                nc.scalar.activation(r_col, s_col, Act.Exp, scale=-0.5)
                negr = sb(pact, [T, 1], f32, tag="negr")
                nc.vector.tensor_scalar_mul(negr, r_col, -1.0)
                rt_ps = ppt.tile([1, T], f32, tag="t256", name="t256")
                nc.tensor.transpose(rt_ps, r_col, idn[0:T, 0:T])
                r_row = sb(pact, [1, T], f32, tag="r_row")
                nc.vector.tensor_copy(out=r_row, in_=rt_ps)
                ps_r = pps.tile([128, T], f32, tag="t128", name="t128")
                nc.tensor.matmul(ps_r, ones1, r_row, start=True, stop=True)
                r_sb = sb(pact, [128, T], f32, tag="r_sb")
                nc.vector.tensor_copy(out=r_sb, in_=ps_r)

                def usrc(k):
                    return prev_c2[:, k * T:(k + 1) * T]
            else:
                def usrc(k):
                    return uTb[:, k, :]

            # z + dt_raw (token-major): [32, 260]
            ps_zd = ppt.tile([T, 260], f32, tag="t256", name="t256")
            for k in range(8):
                nc.tensor.matmul(ps_zd, usrc(k), winzd[:, k, :],
                                 start=(k == 0), stop=(k == 7))
            # read z/dt out of PSUM early to free the slot
            dtt = sb(psm, [T, HC], f32, tag="dtt")
            if l > 0:
                nc.vector.scalar_tensor_tensor(
                    dtt, ps_zd[:, 256:260], r_col, cl["dtbias"],
                    op0=Alu.mult, op1=Alu.add)
            else:
                nc.vector.tensor_tensor(dtt, ps_zd[:, 256:260], cl["dtbias"],
                                        Alu.add)
            zeg = sb(pact, [T, XC], f32, tag="zeg")
            nc.scalar.activation(zeg, ps_zd[:, 0:XC], Act.Exp,
                                 scale=(negr if l > 0 else -1.0))
            za1 = sb(pact, [T, XC], f32, tag="za1")
            nc.vector.tensor_scalar_add(za1, zeg, 1.0)
            zrg = sb(pact, [T, XC], f32, tag="zrg")
            nc.vector.reciprocal_approx_fast(zrg, za1)
            sz = sb(pact, [T, XC], f32, tag="sz")
            if l > 0:
                nc.vector.scalar_tensor_tensor(
                    sz, ps_zd[:, 0:XC], r_col, zrg, op0=Alu.mult, op1=Alu.mult)
            else:
                nc.vector.tensor_tensor(sz, ps_zd[:, 0:XC], zrg, Alu.mult)
            # x/B/C feature-major into ONE [128, 4*T] PSUM tile
            ps_xbc = pps.tile([128, 4 * T], f32, tag="t128", name="t128")
            for ft in range(4):
                for k in range(8):
                    nc.tensor.matmul(
                        ps_xbc[:, ft * T:(ft + 1) * T],
                        winxbc[:, k, ft * 128:(ft + 1) * 128],
                        usrc(k), start=(k == 0), stop=(k == 7))
            # causal depthwise conv: pad 3 tokens per ft, batched taps
            xpad = pact.tile([128, 4, 3 + T], f32, tag="xpad", name="xpad",
                             bufs=2)
            nc.vector.memset(xpad[:, :, 0:3], 0.0)
            if l > 0:
                rsb_b4 = bass.AP(tensor=r_sb.tensor, offset=r_sb.offset,
                                 ap=[list(r_sb.ap[0]), [0, 4],
                                     list(r_sb.ap[1])])
                nc.vector.tensor_tensor(
                    xpad[:, :, 3:3 + T],
                    ps_xbc.rearrange("p (ft t) -> p ft t", ft=4),
                    rsb_b4, Alu.mult)
            else:
                nc.vector.tensor_copy(
                    out=xpad[:, :, 3:3 + T],
                    in_=ps_xbc.rearrange("p (ft t) -> p ft t", ft=4))
            cw3 = cl["cwb"][3].rearrange("p (ft t) -> p ft t", ft=4)
            m3t = sb(pact, [128, 4 * T], f32, tag="cm3")
            nc.vector.tensor_tensor(
                m3t.rearrange("p (ft t) -> p ft t", ft=4),
                xpad[:, :, 3:3 + T], cw3, Alu.mult)
            acc = m3t
            for k in range(3):
                cwk = cl["cwb"][k].rearrange("p (ft t) -> p ft t", ft=4)
                mk = sb(pact, [128, 4 * T], f32, tag=f"cm{k}")
                nc.vector.tensor_tensor(
                    mk.rearrange("p (ft t) -> p ft t", ft=4),
                    xpad[:, :, k:k + T], cwk, Alu.mult)
                a2 = sb(pact, [128, 4 * T], f32, tag=f"ca{k}")
                nc.vector.tensor_tensor(a2, acc, mk, Alu.add)
                acc = a2
            conv = sb(pact, [128, 4 * T], f32, tag="conv")
            nc.vector.tensor_tensor(conv, acc, cl["cbb"], Alu.add)
            # batched SiLU over all of x/B/C (feature-major)
            ceg = sb(pact, [128, 4 * T], f32, tag="ceg")
            nc.scalar.activation(ceg, conv, Act.Exp, scale=-1.0)
            ca1 = sb(pact, [128, 4 * T], f32, tag="ca1")
            nc.vector.tensor_scalar_add(ca1, ceg, 1.0)
            crg = sb(pact, [128, 4 * T], f32, tag="crg")
            nc.vector.reciprocal_approx_fast(crg, ca1)
            xs = sb(pact, [128, 4 * T], f32, tag="xs")
            nc.vector.tensor_tensor(xs, conv, crg, Alu.mult)
            # X token-major [32, 256]
            X_tm = sb(pact, [T, XC], f32, tag="X_tm")
            for i in range(2):
                pt = ppt.tile([T, 128], f32, tag="t256", name="t256")
                nc.tensor.transpose(pt, xs[:, i * T:(i + 1) * T], idn)
                nc.vector.tensor_copy(out=X_tm[:, i * 128:(i + 1) * 128],
                                      in_=pt)
            # GT[s,t] = B^T C  (shared across heads)
            ps_gt = ppt.tile([T, T], f32, tag="t256", name="t256")
            nc.tensor.matmul(ps_gt, xs[:, 2 * T:3 * T], xs[:, 3 * T:4 * T],
                             start=True, stop=True)
            GT = sb(psm, [T, T], f32, tag="GT")
            nc.vector.tensor_copy(out=GT, in_=ps_gt)
            # dt path: dt_tp and n_tp side by side for a joint transpose
            # (dt first so dt_fm lands at base partition 0 for the matmul)
            nd = sb(psm, [T, 2 * HC], f32, tag="nd")
            dte = sb(psm, [T, HC], f32, tag="dte")
            nc.scalar.activation(dte, dtt, Act.Exp)
            nc.scalar.activation(nd[:, 0:HC], dte, Act.Ln, bias=1.0)
            dt_tp = nd[:, 0:HC]
            logdA = sb(psm, [T, HC], f32, tag="logdA")
            nc.vector.tensor_tensor(logdA, dt_tp, cl["abc"], Alu.mult)
            ps_n = ppt.tile([T, HC], f32, tag="t256", name="t256")
            nc.tensor.matmul(ps_n, negut, logdA, start=True, stop=True)
            nc.vector.tensor_copy(out=nd[:, HC:2 * HC], in_=ps_n)
            n_tp = nd[:, HC:2 * HC]
            ps_dtf = ppt.tile([HC, T], f32, tag="t256", name="t256")
            nc.tensor.transpose(ps_dtf, nd[:, 0:HC], idn[0:T, 0:T])
            dt_fm = sb(psm, [HC, T], f32, tag="dt_fm")
            nc.vector.tensor_copy(out=dt_fm, in_=ps_dtf)
            ps_nf = ppt.tile([HC, T], f32, tag="t256", name="t256")
            nc.tensor.transpose(ps_nf, nd[:, HC:2 * HC], idn[0:T, 0:T])
            n_fm = sb(psm, [HC, T], f32, tag="n_fm")
            nc.vector.tensor_copy(out=n_fm, in_=ps_nf)
            # batched SSD across heads: ARG[s, h*T+t] = n_tp[s,h] - n_fm[h,t]
            nfm_b = bass.AP(tensor=n_fm.tensor, offset=n_fm.offset,
                            ap=[list(n_fm.ap[0]), [0, HC], list(n_fm.ap[1])])
            tmpR = sb(psm, [HC, HC * T], f32, tag="tmpR")
            nc.vector.scalar_tensor_tensor(
                tmpR.rearrange("p (h t) -> p h t", h=HC),
                nfm_b, -1.0,
                onehots.rearrange("p (h t) -> p h t", h=HC),
                op0=Alu.mult, op1=Alu.mult)
            ps_arg = ppt.tile([T, HC * T], f32, tag="t256", name="t256")
            nc.tensor.matmul(ps_arg, ones4, tmpR, start=True, stop=False)
            nc.tensor.matmul(ps_arg, n_fm, onehots, start=False, stop=True)
            ps_dtm = ppt.tile([T, HC * T], f32, tag="t256", name="t256")
            nc.tensor.matmul(ps_dtm, dt_fm, onehots, start=True, stop=True)
            arg2 = sb(psm, [T, HC * T], f32, tag="arg2")
            nc.vector.tensor_tensor(arg2, ps_arg, maskrep, Alu.add)
            E_all = sb(psm, [T, HC * T], f32, tag="E_all")
            nc.scalar.activation(E_all, arg2, Act.Exp)
            GT_b = bass.AP(tensor=GT.tensor, offset=GT.offset,
                           ap=[list(GT.ap[0]), [0, HC], list(GT.ap[1])])
            M1 = sb(psm, [T, HC * T], f32, tag="M1")
            nc.vector.tensor_tensor(
                M1.rearrange("p (h t) -> p h t", h=HC),
                E_all.rearrange("p (h t) -> p h t", h=HC), GT_b, Alu.mult)
            M2 = sb(psm, [T, HC * T], f32, tag="M2")
            nc.vector.tensor_tensor(M2, M1, ps_dtm, Alu.mult)
            M3 = sb(psm, [T, HC * T], f32, tag="M3")
            nc.vector.tensor_tensor(M3, M2, cl["ddiag"], Alu.add)
            ps_y = pyacc.tile([T, XC], f32, tag="yacc", name="yacc")
            for h in range(HC):
                nc.tensor.matmul(ps_y[:, h * HD:(h + 1) * HD],
                                 M3[:, h * T:(h + 1) * T],
                                 X_tm[:, h * HD:(h + 1) * HD],
                                 start=True, stop=True)
            # gate + sum of squares
            yg = sb(pact, [T, XC], f32, tag="yg")
            nc.vector.tensor_tensor(yg, ps_y, sz, Alu.mult)
            comb = sb(pact, [128, 257], wdt, tag="comb")
            sq = sb(pact, [T, XC], f32, tag="sq")
            with nc.allow_low_precision(reason="sumsq stat rides bf16 AllReduce"):
                nc.scalar.activation(sq, yg, Act.Square,
                                     accum_out=comb[0:T, 256:257])
            # yg^T -> [128, 2, 32] bf16
            ygT = sb(pact, [128, 2, T], wdt, tag="ygT")
            for i in range(2):
                pt = pps.tile([128, T], f32, tag="t128", name="t128")
                nc.tensor.transpose(pt, yg[:, i * 128:(i + 1) * 128],
                                    idn[0:T, 0:T])
                nc.vector.tensor_copy(out=ygT[:, i, :], in_=pt)
            # partial Wout product, feature-major [128, 8, 32]
            ps_p = pyacc.tile([128, 8, T], f32, tag="yacc", name="yacc")
            for mt in range(8):
                for ki in range(2):
                    nc.tensor.matmul(ps_p[:, mt, :], woutsb[:, ki, mt, :],
                                     ygT[:, ki, :],
                                     start=(ki == 0), stop=(ki == 1))
            nc.vector.tensor_copy(out=comb[:, 0:256], in_=ps_p)
            # fused AllReduce (bf16): [128, 256] partial + [32] sumsq
            arin = dram.tile([128, 257], wdt, tag="arin", name="arin")
            nc.sync.dma_start(out=arin[:], in_=comb)
            arout = dram.tile([128, 257], wdt, tag="arout", name="arout")
            _collective("AllReduce", Alu.add, [arin[:].opt()], [arout[:].opt()])
            warm(34)
            comb2 = sb(pact, [128, 257], wdt, tag="comb2")
            if l < NL - 1:
                # stat column first (tiny) so the rms chain overlaps the
                # bulk transfer; bulk split across two queues so the next
                # layer's first matmul chunks start as soon as they land.
                nc.sync.dma_start(out=comb2[:, 256:257],
                                  in_=arout[:][:, 256:257])
                nc.scalar.dma_start(out=comb2[:, 0:128],
                                    in_=arout[:][:, 0:128])
                nc.gpsimd.dma_start(out=comb2[:, 128:256],
                                    in_=arout[:][:, 128:256])
                prev_c2 = comb2
            else:
                nc.sync.dma_start(out=comb2, in_=arout[:])
                u_sum = comb2[:, 0:256].rearrange("p (k t) -> p k t", k=8)
                s_col = sb(pact, [T, 1], f32, tag="s_col")
                nc.scalar.activation(s_col, comb2[0:T, 256:257], Act.Ln,
                                     bias=eps_t[0:T], scale=1.0 / DI)
                r_col = sb(pact, [T, 1], f32, tag="r_col")
                nc.scalar.activation(r_col, s_col, Act.Exp, scale=-0.5)
                rt_ps = ppt.tile([1, T], f32, tag="t256", name="t256")
                nc.tensor.transpose(rt_ps, r_col, idn[0:T, 0:T])
                r_row = sb(pact, [1, T], f32, tag="r_row")
                nc.vector.tensor_copy(out=r_row, in_=rt_ps)
                ps_r = pps.tile([128, T], f32, tag="t128", name="t128")
                nc.tensor.matmul(ps_r, ones1, r_row, start=True, stop=True)
                ps_r_b = bass.AP(tensor=ps_r.tensor, offset=ps_r.offset,
                                 ap=[list(ps_r.ap[0]), [0, 8],
                                     list(ps_r.ap[1])])
                u8 = sb(pact, [128, 8 * T], dt8, tag="u8")
                with nc.allow_low_precision(reason="stage C fp8 activations"):
                    nc.vector.tensor_tensor(
                        u8.rearrange("p (k t) -> p k t", k=8), u_sum, ps_r_b,
                        Alu.mult)

        # ---- stage C: g = relu(w3 @ flat + b3), fp8 matvec -> AllGather
        # 4 round-robin PSUM accumulators avoid back-to-back accumulation
        # stalls on one region; summed at the end.
        ps_g = pmv.tile([1, 512], f32, tag="mv", name="mv")
        for gchunk in range(8):
            w3sb = sb(pw3, [128, 32, 128], dt8, tag="w3")
            i_d = nc.scalar.dma_start(out=w3sb, in_=w3dr_d[gchunk])
            add_dep_helper(i_d.ins, i_gate_l0.ins, reason="pace w3 after layer0 start")
            for i in range(32):
                cc = gchunk * 32 + i
                t, k = cc // 8, cc % 8
                s = cc % 4
                nc.tensor.matmul(ps_g[0:1, s * 128:(s + 1) * 128],
                                 u8[:, k * T + t:k * T + t + 1],
                                 w3sb[:, i, :],
                                 start=(cc < 4), stop=(cc >= 252))
        gs0 = sb(pact1, [1, 128], f32, tag="gs0")
        nc.vector.tensor_tensor(gs0, ps_g[0:1, 0:128], b3row, Alu.add)
        gs1 = sb(pact1, [1, 128], f32, tag="gs1")
        nc.vector.tensor_tensor(gs1, gs0, ps_g[0:1, 128:256], Alu.add)
        gs2 = sb(pact1, [1, 128], f32, tag="gs2")
        nc.vector.tensor_tensor(gs2, gs1, ps_g[0:1, 256:384], Alu.add)
        g_tmp = sb(pact1, [1, 128], f32, tag="g_tmp")
        nc.vector.tensor_tensor(g_tmp, gs2, ps_g[0:1, 384:512], Alu.add)
        g_act = sb(pact, [1, 128], f32, tag="g_act")
        nc.scalar.activation(g_act, g_tmp, Act.Relu)
        gin = dram.tile([1, 128], f32, tag="gin", name="gin")
        nc.sync.dma_start(out=gin[:], in_=g_act)
        gout = dram.tile([NCORES, 128], f32, tag="gout", name="gout")
        _collective("AllGather", Alu.bypass, [gin[:].opt()], [gout[:].opt()])
        warm(10)
        g_r = sb(pact, [NCORES, 128], f32, tag="g_r")
        nc.sync.dma_start(out=g_r, in_=gout[:])
        ps_gt8 = pps.tile([128, 128], f32, tag="t128", name="t128")
        nc.tensor.transpose(ps_gt8[:, 0:NCORES], g_r, idn[0:NCORES, 0:NCORES])
        g8 = sb(pact, [128, NCORES], dt8, tag="g8")
        with nc.allow_low_precision(reason="stage D fp8 activations"):
            nc.vector.tensor_copy(out=g8, in_=ps_gt8[:, 0:NCORES])

        # ---- stage D: out = sigmoid(w4 @ g + b4), fp8 DoubleRow, column out
        ps_o = pcol.tile([128, 16], f32, tag="gcol", name="gcol")
        for nt in range(4):
            w4sb = sb(pw4, [128, 4, 4, 2, 128], dt8, tag="w4")
            i_d = nc.scalar.dma_start(out=w4sb, in_=w4dr_d[nt])
            add_dep_helper(i_d.ins, i_gate_l0.ins, reason="pace w4 after layer0 start")
            for c4 in range(4):
                cb = nt * 4 + c4
                for pr in range(4):
                    nc.tensor.matmul(
                        ps_o[:, cb:cb + 1], w4sb[:, c4, pr],
                        g8[:, 2 * pr:2 * pr + 2].rearrange(
                            "p (k o) -> p k o", k=2),
                        start=(pr == 0), stop=(pr == 3),
                        perf_mode=mybir.MatmulPerfMode.DoubleRow)
        ob = sb(pact1, [128, 16], f32, tag="ob")
        nc.vector.tensor_tensor(ob, ps_o, b4mat, Alu.add)
        oeg = sb(pact, [128, 16], f32, tag="oeg")
        nc.scalar.activation(oeg, ob, Act.Exp, scale=-1.0)
        oa1 = sb(pact, [128, 16], f32, tag="oa1")
        nc.vector.tensor_scalar_add(oa1, oeg, 1.0)
        out_sb = sb(pact1, [128, 16], f32, tag="out_sb")
        nc.vector.reciprocal_approx_fast(out_sb, oa1)
        nc.sync.dma_start(out=out_d, in_=out_sb)

    nc.compile()
    return nc


_CACHE = {}


def _get_program():
    if "nc" not in _CACHE:
        _CACHE["nc"] = _build_program(BF16)
    return _CACHE["nc"]


def kernel(**inputs):
    from concourse.bass_utils import run_bass_kernel_spmd
    nc = _get_program()
    in_maps = [_prep_core(inputs, c, BF16) for c in range(NCORES)]
    res = run_bass_kernel_spmd(nc, in_maps, core_ids=list(range(NCORES)))
    out = np.concatenate([res.results[c]["out"].T.ravel()
                          for c in range(NCORES)])
    return out.reshape(8, 32, 64).astype(np.float32)


if __name__ == "__main__":
    d = np.load("/tmp/inp.npz")
    inp = {k: d[k] for k in d.files}
    got = kernel(**inp)
    want = np.load("/tmp/want64.npy")
    err = np.abs(got - want) / (np.abs(want) + 1e-6)
    print(f"maxrel {err.max():.3e} mean {err.mean():.3e}")



# revision 42
# speedup vs baseline: 1.0769x; 1.0769x over previous
"""Trainium2 Bass kernel for nn_NeuralControlActor (batch-1 MLP + 4x Mamba2 + MLP).

Tensor-parallel over 8 NeuronCores:
  - mlp_in W1/W2 row-sharded (+AllGather), Mamba2 heads sharded 4/core
    (B/C/conv replicated), gated-RMSNorm stat + Wout partial fused into one
    AllReduce per layer, mlp_out w3/w4 row-sharded (+AllGather).
  - The T=32 sequential scan is computed in closed form (SSD):
    Y_h = (exp(S_t - S_s) o (B^T C) o dt_s + D_h I)^T @ X_h  -- all matmuls,
    batched across the 4 local heads via broadcast matmuls ([32, 128] tiles).
  - Big weights stream as fp8e4 (host-cast); SSD/norm math stays f32.
  - Perf: reciprocal_approx_fast for all sigmoid/silu denominators; depthwise
    conv as batched [128,128] tap ops with host-broadcast weights; the rms
    scale of layer l is folded into layer l+1's elementwise ops so the next
    layer's matmuls read the AllReduce result (comb2) directly; stage D runs
    fp8 DoubleRow with a column-major [128,16] output and batched sigmoid;
    dummy PE matmuls keep the tensor-engine p-state up across collectives;
    full-depth weight pools so w2/w3 prefetch entirely under the NRT entry
    barrier / layer ARs.

Self-contained: hardcodes all shapes; host prep is pure numpy.
"""
import os
import numpy as np
import ml_dtypes

# 66KB AllReduces pick RDH (3-stage) by default; Mesh is lower-latency at
# this size.  Must be set before the NRT builds its communicator.
os.environ.setdefault("NEURON_RT_DBG_RDH_CC", "0")

S, H, T, NL = 4096, 1024, 32, 4
DI, DS, DC, HD, NH = 2048, 128, 4, 64, 32
CD = DI + 2 * DS
DIP = 2 * DI + 2 * DS + NH
ASIZE = 16384
NCORES = 8
HC = NH // NCORES            # 4 heads/core
XC = HC * HD                 # 256 x/z channels per core
MASK_POS = 1000.0
BF16 = ml_dtypes.bfloat16
FP8 = ml_dtypes.float8_e4m3

# ----------------------------------------------------------------- host prep

def _chunkT(w):
    """w [F, K] -> arr [128, K//128, F]: arr[p, k, f] = w[f, k*128+p]."""
    F, K = w.shape
    kc = K // 128
    return np.ascontiguousarray(w.T.reshape(kc, 128, F).transpose(1, 0, 2))


def _prep_consts():
    f32 = np.float32
    c = {}
    ut = np.triu(np.ones((T, T), f32))
    c["negut"] = np.ascontiguousarray(-ut)
    c["maskpos"] = np.ascontiguousarray((np.tril(np.ones((T, T), f32), -1)
                                         * MASK_POS).astype(f32))
    oh = np.zeros((HC, HC * T), f32)
    for h in range(HC):
        oh[h, h * T:(h + 1) * T] = 1.0
    c["onehots"] = oh
    c["ones1"] = np.ones((1, 128), f32)
    # -maskpos replicated per head block: [T, HC*T]
    c["maskrep"] = np.ascontiguousarray(
        np.tile(-c["maskpos"], (1, HC)))
    return c


def _prep_core(inp, c, wdt):
    f32 = np.float32
    m = {}
    x = np.asarray(inp["x"], f32)
    m["x_sb"] = np.ascontiguousarray(x.reshape(32, 128).T).astype(wdt)

    w1c = np.asarray(inp["w1"], f32)[c * 128:(c + 1) * 128]
    m["w1t"] = _chunkT(w1c).astype(FP8)                          # [128,32,128]
    m["b1row"] = np.asarray(inp["b1"], f32)[None, c * 128:(c + 1) * 128]

    w2c = np.asarray(inp["w2"], f32)[c * 4096:(c + 1) * 4096]
    w2t = _chunkT(w2c)                                           # [128,8,4096]
    m["w2t"] = np.ascontiguousarray(
        w2t.reshape(128, 8, 8, 512).transpose(2, 0, 1, 3)).astype(FP8)
    m["b2row"] = np.asarray(inp["b2"], f32)[None, c * 4096:(c + 1) * 4096]

    for l in range(NL):
        win = np.asarray(inp["m_Win"], f32)[l]
        zrows = win[c * XC:(c + 1) * XC]
        dtrows = win[2 * DI + 2 * DS + c * HC:2 * DI + 2 * DS + (c + 1) * HC]
        m[f"winzd{l}"] = _chunkT(np.concatenate([zrows, dtrows], 0)).astype(FP8)
        xrows = win[DI + c * XC:DI + (c + 1) * XC]
        brows = win[2 * DI:2 * DI + DS]
        crows = win[2 * DI + DS:2 * DI + 2 * DS]
        m[f"winxbc{l}"] = _chunkT(
            np.concatenate([xrows, brows, crows], 0)).astype(FP8)  # [128,8,512]

        cw = np.asarray(inp["m_convw"], f32)[l]
        cb = np.asarray(inp["m_convb"], f32)[l]
        chsel = np.concatenate([
            np.arange(c * XC, (c + 1) * XC),
            np.arange(DI, DI + DS),
            np.arange(DI + DS, DI + 2 * DS)])
        cwp = cw[chsel].reshape(4, 128, 4).transpose(1, 0, 2)    # [p, ft, k]
        # broadcast tap weights over tokens: [128, k, ft, T] -> [128, 4*128]
        cwb = np.broadcast_to(
            cwp.transpose(0, 2, 1)[:, :, :, None],
            (128, DC, 4, T)).reshape(128, DC * 4 * T)
        m[f"cwb{l}"] = np.ascontiguousarray(cwb)                 # [128, 512]
        cbb = np.broadcast_to(
            cb[chsel].reshape(4, 128).T[:, :, None],
            (128, 4, T)).reshape(128, 4 * T)
        m[f"cbb{l}"] = np.ascontiguousarray(cbb)                 # [128, 128]

        dtb = np.asarray(inp["m_dtbias"], f32)[l, c * HC:(c + 1) * HC]
        m[f"dtbias{l}"] = np.broadcast_to(dtb[None, :], (T, HC)).copy()
        A = -np.exp(np.asarray(inp["m_Alog"], f32)[l, c * HC:(c + 1) * HC])
        m[f"abc{l}"] = np.broadcast_to(A[None, :], (T, HC)).copy()
        Dv = np.asarray(inp["m_D"], f32)[l, c * HC:(c + 1) * HC]
        dd = np.zeros((T, HC, T), f32)
        for h in range(HC):
            dd[:, h, :] = np.eye(T, dtype=f32) * Dv[h]
        m[f"ddiag{l}"] = dd

        woutp = (np.asarray(inp["m_Wout"], f32)[l]
                 * np.asarray(inp["m_normw"], f32)[l][None, :])
        wc = woutp[:, c * XC:(c + 1) * XC]
        m[f"wout{l}"] = np.ascontiguousarray(
            wc.T.reshape(2, 128, 8, 128).transpose(1, 0, 2, 3)).astype(FP8)

    w3c = np.asarray(inp["w3"], f32)[c * 128:(c + 1) * 128]
    w3t = _chunkT(w3c)                                           # [128,256,128]
    m["w3dr"] = np.ascontiguousarray(
        w3t.reshape(128, 8, 32, 128).transpose(1, 0, 2, 3)).astype(FP8)
    m["b3row"] = np.asarray(inp["b3"], f32)[None, c * 128:(c + 1) * 128]
    w4c = np.asarray(inp["w4"], f32)[c * 2048:(c + 1) * 2048]
    # stationary-weights DoubleRow: w4dr[nt, p, c4, pr, i, j]
    #   = w4c[(4nt+c4)*128 + j, (2pr+i)*128 + p]
    w4v = w4c.reshape(16, 128, 8, 128)            # [c, j, kc, p]
    w4v = w4v.reshape(16, 128, 4, 2, 128)         # [c, j, pr, i, p]
    w4dr = w4v.transpose(4, 0, 2, 3, 1)           # [p, c, pr, i, j]
    w4dr = w4dr.reshape(128, 4, 4, 4, 2, 128).transpose(1, 0, 2, 3, 4, 5)
    m["w4dr"] = np.ascontiguousarray(w4dr).astype(FP8)
    m["b4row"] = np.asarray(inp["b4"], f32)[None, c * 2048:(c + 1) * 2048]
    m["b3col"] = np.ascontiguousarray(
        np.asarray(inp["b3"], f32)[c * 128:(c + 1) * 128][:, None])
    m["b4mat"] = np.ascontiguousarray(
        np.asarray(inp["b4"], f32)[c * 2048:(c + 1) * 2048]
        .reshape(16, 128).T)                      # [p, cblk]

    # packed consts: g32 [32, W], g1 [1, W], gcw [128, W]
    cst = _prep_consts()
    g32 = [cst["negut"], cst["maskpos"],
           np.concatenate([cst["onehots"],
                           np.zeros((T - HC, HC * T), f32)], 0),
           cst["maskrep"]]
    for l in range(NL):
        g32 += [m.pop(f"dtbias{l}"), m.pop(f"abc{l}"),
                m.pop(f"ddiag{l}").reshape(T, HC * T)]
    m["g32"] = np.concatenate(g32, 1)                       # [32, 832]
    g1 = [cst["ones1"], m.pop("b1row"), m.pop("b3row"),
          m.pop("b2row"), m.pop("b4row")]
    m["g1"] = np.concatenate(g1, 1)                         # [1, 6528]
    gcw = []
    for l in range(NL):
        gcw += [m.pop(f"cwb{l}"), m.pop(f"cbb{l}")]
    gcw += [m.pop("b3col"), m.pop("b4mat")]
    m["gcw"] = np.concatenate(gcw, 1)                       # [128, 2577]
    return m


# ------------------------------------------------------------- bass program

def _build_program(wdt_np, collectives=True):
    from contextlib import ExitStack
    import concourse.bacc as bacc
    import concourse.tile as tile
    import concourse.bass as bass
    from concourse import mybir
    from concourse.masks import make_identity

    f32 = mybir.dt.float32
    wdt = mybir.dt.from_np(np.dtype(wdt_np))
    dt8 = mybir.dt.float8e4
    Alu = mybir.AluOpType
    Act = mybir.ActivationFunctionType

    # Force a single ACT table set: every function this kernel uses
    # (exp, ln, relu, square, copy) lives in natural_log_exp_and_others;
    # the default chooser thrashes between exp-only and ln-only sets.
    if not getattr(bacc, "_act_tables_pinned", False):
        _orig_gat = bacc.get_activation_tables

        def _gat(arch):
            t = _orig_gat(arch)
            keep = "natural_log_exp_and_others"
            if keep in t:
                for k in t:
                    if k != keep:
                        t[k] = set()
            return t

        bacc.get_activation_tables = _gat
        bacc._act_tables_pinned = True

    nc = bacc.Bacc("TRN2", target_bir_lowering=False, debug=False,
                   num_devices=NCORES)

    def din(name, shape, dt=wdt):
        return nc.dram_tensor(name, list(shape), dt, kind="ExternalInput").ap()

    # inputs (names must match the per-core map)
    x_sb_d = din("x_sb", [128, 32])
    w1t_d = din("w1t", [128, 32, 128], dt8)
    w2t_d = din("w2t", [8, 128, 8, 512], dt8)
    layer_d = []
    for l in range(NL):
        layer_d.append(dict(
            winzd=din(f"winzd{l}", [128, 8, 260], dt8),
            winxbc=din(f"winxbc{l}", [128, 8, 512], dt8),
            wout=din(f"wout{l}", [128, 2, 8, 128], dt8),
        ))
    g32_d = din("g32", [T, 864], f32)
    g1_d = din("g1", [1, 6528], f32)
    gcw_d = din("gcw", [128, 2577], f32)
    w3dr_d = din("w3dr", [8, 128, 32, 128], dt8)
    w4dr_d = din("w4dr", [4, 128, 4, 4, 2, 128], dt8)
    out_d = nc.dram_tensor("out", [128, 16], f32, kind="ExternalOutput").ap()

    RG = [list(range(NCORES))]

    def _collective(kind, op, ins, outs):
        if collectives:
            nc.gpsimd.collective_compute(kind, op, replica_groups=RG,
                                         ins=ins, outs=outs)
        else:
            nc.sync.dma_start(out=outs[0][0:1], in_=ins[0][0:1])

    from concourse.tile import add_dep_helper

    with tile.TileContext(nc) as tc, ExitStack() as ctx:
        ep = ctx.enter_context
        consts = ep(tc.tile_pool(name="consts", bufs=1))
        pw1 = ep(tc.tile_pool(name="pw1", bufs=1))
        pw2 = ep(tc.tile_pool(name="pw2", bufs=8))
        pwin = ep(tc.tile_pool(name="pwin", bufs=2))
        pw3 = ep(tc.tile_pool(name="pw3", bufs=8))
        pw4 = ep(tc.tile_pool(name="pw4", bufs=4))
        pact = ep(tc.tile_pool(name="pact", bufs=2))
        pact1 = ep(tc.tile_pool(name="pact1", bufs=1))
        psm = ep(tc.tile_pool(name="psm", bufs=2))
        pmv = ep(tc.tile_pool(name="pmv", bufs=2, space="PSUM"))
        pcol = ep(tc.tile_pool(name="pcol", bufs=1, space="PSUM"))
        pps = ep(tc.tile_pool(name="pps", bufs=2, space="PSUM"))
        ppt = ep(tc.tile_pool(name="ppt", bufs=2, space="PSUM"))
        pyacc = ep(tc.tile_pool(name="pyacc", bufs=1, space="PSUM"))
        dram = ep(tc.tile_pool(name="dram", bufs=2, space="DRAM"))

        def sb(pool, shape, dt=f32, tag=None):
            return pool.tile(list(shape), dt, tag=tag, name=tag)

        # ---- constants into SBUF (x/w1 first: stage A is the launch path)
        x_sb = sb(consts, [128, 32], wdt, tag="x_sb")
        nc.sync.dma_start(out=x_sb, in_=x_sb_d)
        w1sb = sb(pw1, [128, 32, 128], dt8, tag="w1")
        nc.sync.dma_start(out=w1sb, in_=w1t_d)
        idn = sb(consts, [128, 128], f32, tag="idn")
        make_identity(nc, idn)
        eps_t = sb(consts, [128, 1], f32, tag="eps_t")
        nc.vector.memset(eps_t, 1e-5)
        g32 = sb(consts, [T, 864], f32, tag="g32")
        nc.sync.dma_start(out=g32, in_=g32_d)
        g1 = sb(consts, [1, 6528], f32, tag="g1")
        nc.sync.dma_start(out=g1, in_=g1_d)
        gcw = sb(consts, [128, 2577], f32, tag="gcw")
        nc.sync.dma_start(out=gcw, in_=gcw_d)
        b3col = gcw[:, 2560:2561]
        b4mat = gcw[:, 2561:2577]
        negut = g32[:, 0:T]
        maskpos = g32[:, T:2 * T]
        onehots = g32[0:HC, 2 * T:2 * T + HC * T]
        maskrep = g32[:, 2 * T + HC * T:2 * T + 2 * HC * T]
        ones1 = g1[0:1, 0:128]
        b1row = g1[0:1, 128:256]
        b3row = g1[0:1, 256:384]
        b2row = g1[0:1, 384:4480]
        b4row = g1[0:1, 4480:6528]
        G32L = 2 * T + 2 * HC * T        # per-layer base in g32
        lc = []
        for l in range(NL):
            base = G32L + l * (2 * HC + HC * T)
            cb = l * 640
            lc.append(dict(
                dtbias=g32[:, base:base + HC],
                abc=g32[:, base + HC:base + 2 * HC],
                ddiag=g32[:, base + 2 * HC:base + 2 * HC + HC * T],
                cwb=[gcw[:, cb + k * 128:cb + (k + 1) * 128] for k in range(4)],
                cbb=gcw[:, cb + 512:cb + 640],
            ))
        # SSD broadcast-matmul helper: ones [4, 32] for row-broadcast
        ones4 = sb(consts, [HC, T], f32, tag="ones4")
        nc.vector.memset(ones4, 1.0)

        # PE warmup bursts: keep HAM busy (and the PE p-state high) while
        # DMAs / collectives are in flight.  Rotates the shared t128 bufs.
        def warm(n):
            for _ in range(n):
                wp = pps.tile([128, 128], f32, tag="t128", name="t128")
                nc.tensor.matmul(wp, idn, idn[:, 0:128],
                                 start=True, stop=True)

        warm(24)

        # ---- stage A: h = relu(W1 x + b1), row shard -> AllGather
        ps_h = pmv.tile([1, 512], f32, tag="mv", name="mv")
        for j in range(32):
            nc.tensor.matmul(ps_h[0:1, 0:128], x_sb[:, j:j + 1], w1sb[:, j, :],
                             start=(j == 0), stop=(j == 31))
        h_tmp = sb(pact, [1, 128], f32, tag="h_tmp")
        i_gate_a = nc.vector.tensor_tensor(h_tmp, ps_h[0:1, 0:128], b1row, Alu.add)
        h_act = sb(pact, [1, 128], f32, tag="h_act")
        nc.scalar.activation(h_act, h_tmp, Act.Relu)
        hin = dram.tile([1, 128], f32, tag="hin", name="hin")
        nc.sync.dma_start(out=hin[:], in_=h_act)
        hout = dram.tile([NCORES, 128], f32, tag="hout", name="hout")
        _collective("AllGather", Alu.bypass, [hin[:].opt()], [hout[:].opt()])
        h_r = sb(pact, [NCORES, 128], f32, tag="h_r")
        nc.sync.dma_start(out=h_r, in_=hout[:])
        ps_ht = pps.tile([128, 128], f32, tag="t128", name="t128")
        nc.tensor.transpose(ps_ht[:, 0:NCORES], h_r, idn[0:NCORES, 0:NCORES])
        h_all = sb(pact, [128, NCORES], wdt, tag="h_all")
        nc.vector.tensor_copy(out=h_all, in_=ps_ht[:, 0:NCORES])

        # ---- stage B: seq shard = W2 h + b2 (4 tokens) -> AllGather
        seq_sb = sb(pact1, [1, 4096], f32, tag="seq_sb")
        for nt in range(8):
            w2sb = sb(pw2, [128, 8, 512], dt8, tag="w2")
            i_d = nc.scalar.dma_start(out=w2sb, in_=w2t_d[nt])
            add_dep_helper(i_d.ins, i_gate_a.ins, reason="pace w2 after stage A")
            ps = pmv.tile([1, 512], f32, tag="mv", name="mv")
            for j in range(8):
                nc.tensor.matmul(ps, h_all[:, j:j + 1], w2sb[:, j, :],
                                 start=(j == 0), stop=(j == 7))
            nc.vector.tensor_tensor(seq_sb[0:1, nt * 512:(nt + 1) * 512], ps,
                                    b2row[0:1, nt * 512:(nt + 1) * 512], Alu.add)
        seqin = dram.tile([1, 4096], f32, tag="seqin", name="seqin")
        nc.sync.dma_start(out=seqin[:], in_=seq_sb)
        seqout = dram.tile([NCORES, 4096], f32, tag="seqout", name="seqout")
        _collective("AllGather", Alu.bypass, [seqin[:].opt()], [seqout[:].opt()])
        warm(20)

        # ---- layer-0 input: load [32,1024] token-major, PE-transpose to uT
        useq = sb(pact1, [T, 1024], f32, tag="useq")
        i_gate_l0 = nc.sync.dma_start(
            out=useq, in_=seqout[:].rearrange("j (a f) -> (j a) f", a=4))
        uTb = sb(pact, [128, 8, T], wdt, tag="uTb")
        for k in range(8):
            pt = pps.tile([128, T], f32, tag="t128", name="t128")
            nc.tensor.transpose(pt, useq[:, k * 128:(k + 1) * 128],
                                idn[0:T, 0:T])
            nc.vector.tensor_copy(out=uTb[:, k, :], in_=pt)

        # ---- Mamba2 layers
        prev_c2 = None
        for l in range(NL):
            ld, cl = layer_d[l], lc[l]
            winzd = sb(pwin, [128, 8, 260], dt8, tag="winzd")
            i_d = nc.gpsimd.dma_start(out=winzd, in_=ld["winzd"])
            add_dep_helper(i_d.ins, i_gate_a.ins, reason="pace win after stage A")
            winxbc = sb(pwin, [128, 8, 512], dt8, tag="winxbc")
            i_d = nc.gpsimd.dma_start(out=winxbc, in_=ld["winxbc"])
            add_dep_helper(i_d.ins, i_gate_a.ins, reason="pace win after stage A")
            woutsb = sb(pwin, [128, 2, 8, 128], dt8, tag="wout")
            i_d = nc.gpsimd.dma_start(out=woutsb, in_=ld["wout"])
            add_dep_helper(i_d.ins, i_gate_a.ins, reason="pace win after stage A")

            if l > 0:
                # rms scale of the previous layer's output, folded downstream
                s_col = sb(pact, [T, 1], f32, tag="s_col")
                nc.scalar.activation(s_col, prev_c2[0:T, 256:257], Act.Ln,
                                     bias=eps_t[0:T], scale=1.0 / DI)
                r_col = sb(pact, [T, 1], f32, tag="r_col")
                nc.scalar.activation(r_col, s_col, Act.Exp, scale=-0.5)
                negr = sb(pact, [T, 1], f32, tag="negr")
                nc.vector.tensor_scalar_mul(negr, r_col, -1.0)
                rt_ps = ppt.tile([1, T], f32, tag="t256", name="t256")
                nc.tensor.transpose(rt_ps, r_col, idn[0:T, 0:T])
                r_row = sb(pact, [1, T], f32, tag="r_row")
                nc.vector.tensor_copy(out=r_row, in_=rt_ps)
                ps_r = pps.tile([128, T], f32, tag="t128", name="t128")
                nc.tensor.matmul(ps_r, ones1, r_row, start=True, stop=True)
                r_sb = sb(pact, [128, T], f32, tag="r_sb")
                nc.vector.tensor_copy(out=r_sb, in_=ps_r)

                def usrc(k):
                    return prev_c2[:, k * T:(k + 1) * T]
            else:
                def usrc(k):
                    return uTb[:, k, :]

            # z + dt_raw (token-major): [32, 260]
            ps_zd = ppt.tile([T, 260], f32, tag="t256", name="t256")
            for k in range(8):
                nc.tensor.matmul(ps_zd, usrc(k), winzd[:, k, :],
                                 start=(k == 0), stop=(k == 7))
            # read z/dt out of PSUM early to free the slot
            dtt = sb(psm, [T, HC], f32, tag="dtt")
            if l > 0:
                nc.vector.scalar_tensor_tensor(
                    dtt, ps_zd[:, 256:260], r_col, cl["dtbias"],
                    op0=Alu.mult, op1=Alu.add)
            else:
                nc.vector.tensor_tensor(dtt, ps_zd[:, 256:260], cl["dtbias"],
                                        Alu.add)
            zeg = sb(pact, [T, XC], f32, tag="zeg")
            nc.scalar.activation(zeg, ps_zd[:, 0:XC], Act.Exp,
                                 scale=(negr if l > 0 else -1.0))
            za1 = sb(pact, [T, XC], f32, tag="za1")
            nc.vector.tensor_scalar_add(za1, zeg, 1.0)
            zrg = sb(pact, [T, XC], f32, tag="zrg")
            nc.vector.reciprocal_approx_fast(zrg, za1)
            sz = sb(pact, [T, XC], f32, tag="sz")
            if l > 0:
                nc.vector.scalar_tensor_tensor(
                    sz, ps_zd[:, 0:XC], r_col, zrg, op0=Alu.mult, op1=Alu.mult)
            else:
                nc.vector.tensor_tensor(sz, ps_zd[:, 0:XC], zrg, Alu.mult)
            # x/B/C feature-major into ONE [128, 4*T] PSUM tile
            ps_xbc = pps.tile([128, 4 * T], f32, tag="t128", name="t128")
            for ft in range(4):
                for k in range(8):
                    nc.tensor.matmul(
                        ps_xbc[:, ft * T:(ft + 1) * T],
                        winxbc[:, k, ft * 128:(ft + 1) * 128],
                        usrc(k), start=(k == 0), stop=(k == 7))
            # causal depthwise conv: pad 3 tokens per ft, batched taps
            xpad = pact.tile([128, 4, 3 + T], f32, tag="xpad", name="xpad",
                             bufs=2)
            if l < 2:   # bufs=2 rotation: pads stay zero afterwards
                nc.vector.memset(xpad[:, :, 0:3], 0.0)
            if l > 0:
                rsb_b4 = bass.AP(tensor=r_sb.tensor, offset=r_sb.offset,
                                 ap=[list(r_sb.ap[0]), [0, 4],
                                     list(r_sb.ap[1])])
                nc.vector.tensor_tensor(
                    xpad[:, :, 3:3 + T],
                    ps_xbc.rearrange("p (ft t) -> p ft t", ft=4),
                    rsb_b4, Alu.mult)
            else:
                nc.vector.tensor_copy(
                    out=xpad[:, :, 3:3 + T],
                    in_=ps_xbc.rearrange("p (ft t) -> p ft t", ft=4))
            cw3 = cl["cwb"][3].rearrange("p (ft t) -> p ft t", ft=4)
            m3t = sb(pact, [128, 4 * T], f32, tag="cm3")
            nc.vector.tensor_tensor(
                m3t.rearrange("p (ft t) -> p ft t", ft=4),
                xpad[:, :, 3:3 + T], cw3, Alu.mult)
            acc = m3t
            for k in range(3):
                cwk = cl["cwb"][k].rearrange("p (ft t) -> p ft t", ft=4)
                mk = sb(pact, [128, 4 * T], f32, tag=f"cm{k}")
                nc.vector.tensor_tensor(
                    mk.rearrange("p (ft t) -> p ft t", ft=4),
                    xpad[:, :, k:k + T], cwk, Alu.mult)
                a2 = sb(pact, [128, 4 * T], f32, tag=f"ca{k}")
                nc.vector.tensor_tensor(a2, acc, mk, Alu.add)
                acc = a2
            conv = sb(pact, [128, 4 * T], f32, tag="conv")
            nc.vector.tensor_tensor(conv, acc, cl["cbb"], Alu.add)
            # batched SiLU over all of x/B/C (feature-major)
            ceg = sb(pact, [128, 4 * T], f32, tag="ceg")
            nc.scalar.activation(ceg, conv, Act.Exp, scale=-1.0)
            ca1 = sb(pact, [128, 4 * T], f32, tag="ca1")
            nc.vector.tensor_scalar_add(ca1, ceg, 1.0)
            crg = sb(pact, [128, 4 * T], f32, tag="crg")
            nc.vector.reciprocal_approx_fast(crg, ca1)
            xs = sb(pact, [128, 4 * T], f32, tag="xs")
            nc.vector.tensor_tensor(xs, conv, crg, Alu.mult)
            # X token-major [32, 256]
            X_tm = sb(pact, [T, XC], f32, tag="X_tm")
            for i in range(2):
                pt = ppt.tile([T, 128], f32, tag="t256", name="t256")
                nc.tensor.transpose(pt, xs[:, i * T:(i + 1) * T], idn)
                nc.vector.tensor_copy(out=X_tm[:, i * 128:(i + 1) * 128],
                                      in_=pt)
            # GT[s,t] = B^T C  (shared across heads)
            ps_gt = ppt.tile([T, T], f32, tag="t256", name="t256")
            nc.tensor.matmul(ps_gt, xs[:, 2 * T:3 * T], xs[:, 3 * T:4 * T],
                             start=True, stop=True)
            GT = sb(psm, [T, T], f32, tag="GT")
            nc.vector.tensor_copy(out=GT, in_=ps_gt)
            # dt path: dt_tp and n_tp side by side for a joint transpose
            # (dt first so dt_fm lands at base partition 0 for the matmul)
            nd = sb(psm, [T, 2 * HC], f32, tag="nd")
            dte = sb(psm, [T, HC], f32, tag="dte")
            nc.scalar.activation(dte, dtt, Act.Exp)
            nc.scalar.activation(nd[:, 0:HC], dte, Act.Ln, bias=1.0)
            dt_tp = nd[:, 0:HC]
            logdA = sb(psm, [T, HC], f32, tag="logdA")
            nc.vector.tensor_tensor(logdA, dt_tp, cl["abc"], Alu.mult)
            ps_n = ppt.tile([T, HC], f32, tag="t256", name="t256")
            nc.tensor.matmul(ps_n, negut, logdA, start=True, stop=True)
            nc.vector.tensor_copy(out=nd[:, HC:2 * HC], in_=ps_n)
            n_tp = nd[:, HC:2 * HC]
            ps_dtf = ppt.tile([HC, T], f32, tag="t256", name="t256")
            nc.tensor.transpose(ps_dtf, nd[:, 0:HC], idn[0:T, 0:T])
            dt_fm = sb(psm, [HC, T], f32, tag="dt_fm")
            nc.vector.tensor_copy(out=dt_fm, in_=ps_dtf)
            ps_nf = ppt.tile([HC, T], f32, tag="t256", name="t256")
            nc.tensor.transpose(ps_nf, nd[:, HC:2 * HC], idn[0:T, 0:T])
            n_fm = sb(psm, [HC, T], f32, tag="n_fm")
            nc.vector.tensor_copy(out=n_fm, in_=ps_nf)
            # batched SSD across heads: ARG[s, h*T+t] = n_tp[s,h] - n_fm[h,t]
            nfm_b = bass.AP(tensor=n_fm.tensor, offset=n_fm.offset,
                            ap=[list(n_fm.ap[0]), [0, HC], list(n_fm.ap[1])])
            tmpR = sb(psm, [HC, HC * T], f32, tag="tmpR")
            nc.vector.scalar_tensor_tensor(
                tmpR.rearrange("p (h t) -> p h t", h=HC),
                nfm_b, -1.0,
                onehots.rearrange("p (h t) -> p h t", h=HC),
                op0=Alu.mult, op1=Alu.mult)
            ps_arg = ppt.tile([T, HC * T], f32, tag="t256", name="t256")
            nc.tensor.matmul(ps_arg, ones4, tmpR, start=True, stop=False)
            nc.tensor.matmul(ps_arg, n_fm, onehots, start=False, stop=True)
            ps_dtm = ppt.tile([T, HC * T], f32, tag="t256", name="t256")
            nc.tensor.matmul(ps_dtm, dt_fm, onehots, start=True, stop=True)
            arg2 = sb(psm, [T, HC * T], f32, tag="arg2")
            nc.vector.tensor_tensor(arg2, ps_arg, maskrep, Alu.add)
            E_all = sb(psm, [T, HC * T], f32, tag="E_all")
            nc.scalar.activation(E_all, arg2, Act.Exp)
            GT_b = bass.AP(tensor=GT.tensor, offset=GT.offset,
                           ap=[list(GT.ap[0]), [0, HC], list(GT.ap[1])])
            M1 = sb(psm, [T, HC * T], f32, tag="M1")
            nc.vector.tensor_tensor(
                M1.rearrange("p (h t) -> p h t", h=HC),
                E_all.rearrange("p (h t) -> p h t", h=HC), GT_b, Alu.mult)
            M2 = sb(psm, [T, HC * T], f32, tag="M2")
            nc.vector.tensor_tensor(M2, M1, ps_dtm, Alu.mult)
            M3 = sb(psm, [T, HC * T], f32, tag="M3")
            nc.vector.tensor_tensor(M3, M2, cl["ddiag"], Alu.add)
            ps_y = pyacc.tile([T, XC], f32, tag="yacc", name="yacc")
            for h in range(HC):
                nc.tensor.matmul(ps_y[:, h * HD:(h + 1) * HD],
                                 M3[:, h * T:(h + 1) * T],
                                 X_tm[:, h * HD:(h + 1) * HD],
                                 start=True, stop=True)
            # gate + sum of squares
            yg = sb(pact, [T, XC], f32, tag="yg")
            nc.vector.tensor_tensor(yg, ps_y, sz, Alu.mult)
            comb = sb(pact, [128, 257], wdt, tag="comb")
            sq = sb(pact, [T, XC], f32, tag="sq")
            with nc.allow_low_precision(reason="sumsq stat rides bf16 AllReduce"):
                nc.scalar.activation(sq, yg, Act.Square,
                                     accum_out=comb[0:T, 256:257])
            # yg^T -> [128, 2, 32] bf16
            ygT = sb(pact, [128, 2, T], wdt, tag="ygT")
            for i in range(2):
                pt = pps.tile([128, T], f32, tag="t128", name="t128")
                nc.tensor.transpose(pt, yg[:, i * 128:(i + 1) * 128],
                                    idn[0:T, 0:T])
                nc.vector.tensor_copy(out=ygT[:, i, :], in_=pt)
            # partial Wout product, feature-major [128, 8, 32]
            ps_p = pyacc.tile([128, 8, T], f32, tag="yacc", name="yacc")
            for mt in range(8):
                for ki in range(2):
                    nc.tensor.matmul(ps_p[:, mt, :], woutsb[:, ki, mt, :],
                                     ygT[:, ki, :],
                                     start=(ki == 0), stop=(ki == 1))
            nc.vector.tensor_copy(out=comb[:, 0:256], in_=ps_p)
            # fused AllReduce (bf16): [128, 256] partial + [32] sumsq
            arin = dram.tile([128, 257], wdt, tag="arin", name="arin")
            nc.sync.dma_start(out=arin[:], in_=comb)
            arout = dram.tile([128, 257], wdt, tag="arout", name="arout")
            _collective("AllReduce", Alu.add, [arin[:].opt()], [arout[:].opt()])
            warm(34)
            comb2 = sb(pact, [128, 257], wdt, tag="comb2")
            if l < NL - 1:
                # stat column first (tiny) so the rms chain overlaps the
                # bulk transfer; bulk split across two queues so the next
                # layer's first matmul chunks start as soon as they land.
                nc.sync.dma_start(out=comb2[:, 256:257],
                                  in_=arout[:][:, 256:257])
                nc.scalar.dma_start(out=comb2[:, 0:128],
                                    in_=arout[:][:, 0:128])
                nc.gpsimd.dma_start(out=comb2[:, 128:256],
                                    in_=arout[:][:, 128:256])
                prev_c2 = comb2
            else:
                # split transfer here too: stat first, halves in parallel,
                # u8 produced per half so stage C starts on the first half.
                nc.sync.dma_start(out=comb2[:, 256:257],
                                  in_=arout[:][:, 256:257])
                nc.scalar.dma_start(out=comb2[:, 0:128],
                                    in_=arout[:][:, 0:128])
                nc.gpsimd.dma_start(out=comb2[:, 128:256],
                                    in_=arout[:][:, 128:256])
                s_col = sb(pact, [T, 1], f32, tag="s_col")
                nc.scalar.activation(s_col, comb2[0:T, 256:257], Act.Ln,
                                     bias=eps_t[0:T], scale=1.0 / DI)
                r_col = sb(pact, [T, 1], f32, tag="r_col")
                nc.scalar.activation(r_col, s_col, Act.Exp, scale=-0.5)
                rt_ps = ppt.tile([1, T], f32, tag="t256", name="t256")
                nc.tensor.transpose(rt_ps, r_col, idn[0:T, 0:T])
                r_row = sb(pact, [1, T], f32, tag="r_row")
                nc.vector.tensor_copy(out=r_row, in_=rt_ps)
                ps_r = pps.tile([128, T], f32, tag="t128", name="t128")
                nc.tensor.matmul(ps_r, ones1, r_row, start=True, stop=True)
                ps_r_b4 = bass.AP(tensor=ps_r.tensor, offset=ps_r.offset,
                                  ap=[list(ps_r.ap[0]), [0, 4],
                                      list(ps_r.ap[1])])
                u8 = sb(pact, [128, 8 * T], dt8, tag="u8")
                with nc.allow_low_precision(reason="stage C fp8 activations"):
                    for hf in range(2):
                        sl = slice(hf * 128, (hf + 1) * 128)
                        nc.vector.tensor_tensor(
                            u8[:, sl].rearrange("p (k t) -> p k t", k=4),
                            comb2[:, sl].rearrange("p (k t) -> p k t", k=4),
                            ps_r_b4, Alu.mult)

        # ---- stage C: g = relu(w3 @ flat + b3), fp8 matvec -> AllGather
        # 4 round-robin PSUM accumulators avoid back-to-back accumulation
        # stalls on one region; summed at the end.
        ps_g = pmv.tile([1, 512], f32, tag="mv", name="mv")
        for gchunk in range(8):
            w3sb = sb(pw3, [128, 32, 128], dt8, tag="w3")
            i_d = nc.scalar.dma_start(out=w3sb, in_=w3dr_d[gchunk])
            add_dep_helper(i_d.ins, i_gate_l0.ins, reason="pace w3 after layer0 start")
            for i in range(32):
                cc = gchunk * 32 + i
                t, k = cc // 8, cc % 8
                s = cc % 4
                nc.tensor.matmul(ps_g[0:1, s * 128:(s + 1) * 128],
                                 u8[:, k * T + t:k * T + t + 1],
                                 w3sb[:, i, :],
                                 start=(cc < 4), stop=(cc >= 252))
        gs0 = sb(pact1, [1, 128], f32, tag="gs0")
        nc.vector.tensor_tensor(gs0, ps_g[0:1, 0:128], b3row, Alu.add)
        gs1 = sb(pact1, [1, 128], f32, tag="gs1")
        nc.vector.tensor_tensor(gs1, gs0, ps_g[0:1, 128:256], Alu.add)
        gs2 = sb(pact1, [1, 128], f32, tag="gs2")
        nc.vector.tensor_tensor(gs2, gs1, ps_g[0:1, 256:384], Alu.add)
        g_tmp = sb(pact1, [1, 128], f32, tag="g_tmp")
        nc.vector.tensor_tensor(g_tmp, gs2, ps_g[0:1, 384:512], Alu.add)
        g_act = sb(pact, [1, 128], f32, tag="g_act")
        nc.scalar.activation(g_act, g_tmp, Act.Relu)
        gin = dram.tile([1, 128], f32, tag="gin", name="gin")
        nc.sync.dma_start(out=gin[:], in_=g_act)
        gout = dram.tile([NCORES, 128], f32, tag="gout", name="gout")
        _collective("AllGather", Alu.bypass, [gin[:].opt()], [gout[:].opt()])
        warm(10)
        g_r = sb(pact, [NCORES, 128], f32, tag="g_r")
        nc.sync.dma_start(out=g_r, in_=gout[:])
        ps_gt8 = pps.tile([128, 128], f32, tag="t128", name="t128")
        nc.tensor.transpose(ps_gt8[:, 0:NCORES], g_r, idn[0:NCORES, 0:NCORES])
        g8 = sb(pact, [128, NCORES], dt8, tag="g8")
        with nc.allow_low_precision(reason="stage D fp8 activations"):
            nc.vector.tensor_copy(out=g8, in_=ps_gt8[:, 0:NCORES])

        # ---- stage D: out = sigmoid(w4 @ g + b4), fp8 DoubleRow, column out
        ps_o = pcol.tile([128, 16], f32, tag="gcol", name="gcol")
        for nt in range(4):
            w4sb = sb(pw4, [128, 4, 4, 2, 128], dt8, tag="w4")
            i_d = nc.scalar.dma_start(out=w4sb, in_=w4dr_d[nt])
            add_dep_helper(i_d.ins, i_gate_l0.ins, reason="pace w4 after layer0 start")
            for c4 in range(4):
                cb = nt * 4 + c4
                for pr in range(4):
                    nc.tensor.matmul(
                        ps_o[:, cb:cb + 1], w4sb[:, c4, pr],
                        g8[:, 2 * pr:2 * pr + 2].rearrange(
                            "p (k o) -> p k o", k=2),
                        start=(pr == 0), stop=(pr == 3),
                        perf_mode=mybir.MatmulPerfMode.DoubleRow)
        ob = sb(pact1, [128, 16], f32, tag="ob")
        nc.vector.tensor_tensor(ob, ps_o, b4mat, Alu.add)
        oeg = sb(pact, [128, 16], f32, tag="oeg")
        nc.scalar.activation(oeg, ob, Act.Exp, scale=-1.0)
        oa1 = sb(pact, [128, 16], f32, tag="oa1")
        nc.vector.tensor_scalar_add(oa1, oeg, 1.0)
        out_sb = sb(pact1, [128, 16], f32, tag="out_sb")
        nc.vector.reciprocal_approx_fast(out_sb, oa1)
        nc.sync.dma_start(out=out_d, in_=out_sb)

    nc.compile()
    return nc


_CACHE = {}


def _get_program():
    if "nc" not in _CACHE:
        _CACHE["nc"] = _build_program(BF16)
    return _CACHE["nc"]


def kernel(**inputs):
    from concourse.bass_utils import run_bass_kernel_spmd
    nc = _get_program()
    in_maps = [_prep_core(inputs, c, BF16) for c in range(NCORES)]
    res = run_bass_kernel_spmd(nc, in_maps, core_ids=list(range(NCORES)))
    out = np.concatenate([res.results[c]["out"].T.ravel()
                          for c in range(NCORES)])
    return out.reshape(8, 32, 64).astype(np.float32)


if __name__ == "__main__":
    d = np.load("/tmp/inp.npz")
    inp = {k: d[k] for k in d.files}
    got = kernel(**inp)
    want = np.load("/tmp/want64.npy")
    err = np.abs(got - want) / (np.abs(want) + 1e-6)
    print(f"maxrel {err.max():.3e} mean {err.mean():.3e}")

